# revision 25
# baseline (speedup 1.0000x reference)
"""Trainium2 Bass kernel for MiMoAudio attention (GQA + neox RoPE + causal softmax + o_proj).

Strategy (tensor-parallel over heads, 8 cores):
  - Each core owns 2 of the 16 q heads (128 q channels) and the single kv head
    (64 channels) that those q heads attend to (GQA group).
  - Host pre-transposes hidden_states to xT [H, B*S]; all activations live
    feature-on-partitions: qT [128, T], kvT [128, T] (k rows 0:64, v 64:128),
    scores [keys=128, queries] per 128-key tile.
  - Per j-tile both heads' scores land in one [128,1024] PSUM tile (the QK
    pair runs concurrently via PE row-tiling at tile_position (0,0)/(64,0)),
    one Exp activation covers both, and the PV pair accumulates into a shared
    [128,1024] po tile (rows 0:64 = out, row 64 = ones-row denominator).
  - Causal masking: query-range restriction per diagonal tile plus a -1e9
    lower-triangle [128,128] constant accumulated into the scores via a tiny
    extra matmul (N=128) before the exp.
  - Softmax denominators: ACT copies the denom row to SBUF, DVE
    reciprocal_approx_fast, broadcast back over 64 partitions via a K=1
    matmul, DVE multiply.
  - o_proj row-slice per core produces a partial [T, H] output; host sums the
    8 partials (the TP all-reduce, done at unshard time).
  - Emission order interleaves projection blocks and the two batches'
    attention blocks so ACT (exp) stays saturated and PE stays warm.
"""

import os
import numpy as np

B = 2
S = 2048
T = B * S
H = 1024
HD = 64
P = 128
NCORES = 8
THETA = 10000.0
SCALE = HD ** -0.5
NBLK = T // 512
HO = H // P
SJT = S // P

_NC_CACHE = {}
LAST_RESULT = None


def _ensure_ntff_hook():
    """Provide antenv.axon_hooks if the image lacks it, so BASS_TRACE=1
    profiling works under axon instead of crashing on import."""
    import sys
    import types

    try:
        import antenv.axon_hooks  # noqa: F401
        return
    except ImportError:
        pass
    mod = types.ModuleType("antenv.axon_hooks")
    mod._hook = None

    def set_axon_ntff_profile_hook(h):
        mod._hook = h

    def get_axon_ntff_profile_hook():
        return mod._hook

    mod.set_axon_ntff_profile_hook = set_axon_ntff_profile_hook
    mod.get_axon_ntff_profile_hook = get_axon_ntff_profile_hook
    sys.modules["antenv.axon_hooks"] = mod
    try:
        import antenv

        antenv.axon_hooks = mod
    except ImportError:
        pass
    try:
        from trn_agent_boot.trn_boot import _ntff_profile_via_ctypes

        hook = _ntff_profile_via_ctypes("/opt/axon/libaxon_pjrt.so")
        if hook is not None:
            mod.set_axon_ntff_profile_hook(hook)
    except Exception:
        pass


_ensure_ntff_hook()


def _build_nc(mm_mode="bf16"):
    import concourse.bass as bass  # noqa: F401
    import concourse.mybir as mybir
    import concourse.tile as tile
    from concourse import bacc

    f32 = mybir.dt.float32
    bf16 = mybir.dt.bfloat16
    Act = mybir.ActivationFunctionType
    recip_exact = os.environ.get("KERNEL_RECIP", "approx") == "exact"

    nc = bacc.Bacc(None, target_bir_lowering=False, debug=False)

    xT_d = nc.dram_tensor("xT", [H, T], bf16, kind="ExternalInput")
    wq_d = nc.dram_tensor("wq", [H, P], bf16, kind="ExternalInput")
    bq_d = nc.dram_tensor("bq", [P, 1], f32, kind="ExternalInput")
    wkv_d = nc.dram_tensor("wkv", [H, P], bf16, kind="ExternalInput")
    bkv_d = nc.dram_tensor("bkv", [P, 1], f32, kind="ExternalInput")
    wo_d = nc.dram_tensor("wo", [P, H], bf16, kind="ExternalInput")
    cos_d = nc.dram_tensor("cosT", [P, S], bf16, kind="ExternalInput")
    sin_d = nc.dram_tensor("sinT", [P, S], bf16, kind="ExternalInput")
    perm_d = nc.dram_tensor("perm", [P, P], bf16, kind="ExternalInput")
    id_d = nc.dram_tensor("ident", [P, P], bf16, kind="ExternalInput")
    mask_d = nc.dram_tensor("maskc", [P, P], bf16, kind="ExternalInput")
    ones_d = nc.dram_tensor("ones64", [1, 64], bf16, kind="ExternalInput")
    onesf_d = nc.dram_tensor("ones64f", [1, 64], f32, kind="ExternalInput")
    onescol_d = nc.dram_tensor("onescol", [P, 2 * SJT], bf16, kind="ExternalInput")
    out_d = nc.dram_tensor("out", [T, H], f32, kind="ExternalOutput")

    with tile.TileContext(nc) as tc:
        with (
            tc.tile_pool(name="const", bufs=1) as cpool,
            tc.tile_pool(name="persist", bufs=1) as ppool,
            tc.tile_pool(name="xt", bufs=3) as xt_pool,
            tc.tile_pool(name="ptile", bufs=4) as p_pool,
            tc.tile_pool(name="ropetmp", bufs=3) as tmp_pool,
            tc.tile_pool(name="atp", bufs=2) as at_pool,
            tc.tile_pool(name="a1p", bufs=2) as a1_pool,
            tc.tile_pool(name="nrm", bufs=2) as nrm_pool,
            tc.tile_pool(name="osb", bufs=3) as osb_pool,
            tc.tile_pool(name="sc_ps", bufs=2, space="PSUM") as sc_pool,
            tc.tile_pool(name="po_ps", bufs=2, space="PSUM") as po_pool,
        ):
            # --- constants (wq/wkv + the first two x-blocks first: they gate
            # the first matmuls; everything else loads behind them) ----------
            wq_sb = cpool.tile([P, HO, P], bf16)
            nc.sync.dma_start(wq_sb[:], wq_d[:].rearrange("(o p) m -> p o m", p=P))
            wkv_sb = cpool.tile([P, HO, P], bf16)
            nc.sync.dma_start(wkv_sb[:], wkv_d[:].rearrange("(o p) m -> p o m", p=P))
            xT_r = xT_d[:].rearrange("(o p) t -> p o t", p=P)
            xt_first = {}
            for blk in (0, 4):
                xt = xt_pool.tile([P, HO, 512], bf16, tag="xt")
                nc.sync.dma_start(
                    xt[:], xT_r[:, :, blk * 512:(blk + 1) * 512]
                )
                xt_first[blk] = xt
            bq_sb = cpool.tile([P, 1], f32)
            nc.sync.dma_start(bq_sb[:], bq_d[:])
            bkv_sb = cpool.tile([P, 1], f32)
            nc.sync.dma_start(bkv_sb[:], bkv_d[:])
            wo_sb = cpool.tile([P, H], bf16)
            nc.sync.dma_start(wo_sb[:], wo_d[:])
            cos_sb = cpool.tile([P, S], bf16)
            nc.sync.dma_start(cos_sb[:], cos_d[:])
            sin_sb = cpool.tile([P, S], bf16)
            nc.sync.dma_start(sin_sb[:], sin_d[:])
            perm_sb = cpool.tile([P, P], bf16)
            nc.sync.dma_start(perm_sb[:], perm_d[:])
            id_sb = cpool.tile([P, P], bf16)
            nc.sync.dma_start(id_sb[:], id_d[:])
            mask_sb = cpool.tile([P, P], bf16)
            nc.sync.dma_start(mask_sb[:], mask_d[:])
            ones_sb = cpool.tile([1, 64], bf16)
            nc.sync.dma_start(ones_sb[:], ones_d[:])
            onesf_sb = cpool.tile([1, 64], f32)
            nc.sync.dma_start(onesf_sb[:], onesf_d[:])

            # --- persistent activations -------------------------------------
            qT = ppool.tile([P, T], bf16)
            kvT = ppool.tile([P, T], bf16)
            khi = ppool.tile([P, T], bf16)
            vnat = ppool.tile([P, 2 * SJT, 72], bf16)
            nc.sync.dma_start(
                vnat[:, :, 64:65], onescol_d[:].rearrange("p (j o) -> p j o", o=1)
            )

            def a_steps(blk):
                """QKV projection + rope + layout prep for one 512-token block,
                as a list of substep closures for interleaving into j-loops.
                All PSUM tiles come from the po-tag ring so the sc-tag ring
                stays dedicated to QK score tiles."""
                tb = slice(blk * 512, (blk + 1) * 512)
                st = {}

                def s1():
                    if blk in xt_first:
                        xt = xt_first.pop(blk)
                    else:
                        xt = xt_pool.tile([P, HO, 512], bf16, tag="xt")
                        nc.sync.dma_start(xt[:], xT_r[:, :, tb])
                    qkv = po_pool.tile([P, 1024], f32, tag="po")
                    for o in range(HO):
                        nc.tensor.matmul(
                            qkv[:, 0:512], wq_sb[:, o, :], xt[:, o, :],
                            start=(o == 0), stop=(o == HO - 1),
                        )
                    st["xt"], st["qkv"] = xt, qkv

                def s2():
                    xt, qkv = st["xt"], st["qkv"]
                    for o in range(HO):
                        nc.tensor.matmul(
                            qkv[:, 512:1024], wkv_sb[:, o, :], xt[:, o, :],
                            start=(o == 0), stop=(o == HO - 1),
                        )
                    nc.vector.tensor_scalar_add(qT[:, tb], qkv[:, 0:512], bq_sb[:])
                    nc.vector.tensor_scalar_add(kvT[:, tb], qkv[:, 512:1024],
                                                bkv_sb[:])

                def s3():
                    sc0 = (blk * 512) % S
                    ss = slice(sc0, sc0 + 512)
                    pr = po_pool.tile([P, 1024], f32, tag="po")
                    nc.tensor.matmul(pr[:, 0:512], perm_sb[:], qT[:, tb],
                                     start=True, stop=True)
                    nc.tensor.matmul(pr[0:64, 512:1024], perm_sb[0:64, 0:64],
                                     kvT[0:64, tb], start=True, stop=True)
                    tmp = tmp_pool.tile([P, 512], bf16, tag="ropetmp")
                    nc.vector.tensor_mul(tmp[:], pr[:, 0:512], sin_sb[:, ss])
                    nc.gpsimd.tensor_mul(qT[:, tb], qT[:, tb], cos_sb[:, ss])
                    nc.gpsimd.tensor_add(qT[:, tb], qT[:, tb], tmp[:])
                    tmpk = tmp_pool.tile([P, 512], bf16, tag="ropetmp")
                    nc.vector.tensor_mul(tmpk[0:64, :], pr[0:64, 512:1024],
                                         sin_sb[0:64, ss])
                    nc.gpsimd.tensor_mul(kvT[0:64, tb], kvT[0:64, tb],
                                         cos_sb[0:64, ss])
                    nc.gpsimd.tensor_add(kvT[0:64, tb], kvT[0:64, tb],
                                         tmpk[0:64, :])
                    nc.sync.dma_start(khi[64:128, tb], kvT[0:64, tb])

                def vsteps(lo, hi):
                    def s():
                        for jj in range(lo, hi):
                            jt = blk * 4 + jj
                            tp = po_pool.tile([P, 1024], bf16, tag="po")
                            nc.tensor.transpose(
                                tp[:, 0:64], kvT[64:128, jt * P:(jt + 1) * P],
                                id_sb[64:128, 64:128],
                            )
                            nc.vector.tensor_copy(vnat[:, jt, 0:64], tp[:, 0:64])
                    return s

                return [s1, s2, s3, vsteps(0, 2), vsteps(2, 4)]

            def emit_B_jl(b, ib, queue):
                """Attention j-loop; pops one interleave substep per j."""
                njt = 4 * (ib + 1)
                q0 = b * S + ib * 512
                po = po_pool.tile([P, 1024], f32, tag="po")
                scs = {}

                def emit_qk(j):
                    qoff = max(0, 128 * (j - 4 * ib))
                    jt = b * SJT + j
                    js = slice(jt * P, (jt + 1) * P)
                    qs = slice(q0 + qoff, q0 + 512)
                    sc_t = sc_pool.tile([P, 1024], f32, tag="sc")
                    diag = j >= 4 * ib
                    if diag:
                        # mask first (start=True clears the bank, writes the
                        # -1e9 triangle); QK then accumulates onto it, so the
                        # exp's dependency ends at the QK matmul itself.
                        nc.tensor.matmul(sc_t[:, qoff:qoff + 128], id_sb[:],
                                         mask_sb[:], start=True, stop=False)
                        nc.tensor.matmul(sc_t[:, 512 + qoff:512 + qoff + 128],
                                         id_sb[:], mask_sb[:],
                                         start=True, stop=False)
                    nc.tensor.matmul(sc_t[:, qoff:512], kvT[0:64, js],
                                     qT[0:64, qs], start=not diag, stop=True)
                    nc.tensor.matmul(sc_t[:, 512 + qoff:1024], khi[64:128, js],
                                     qT[64:128, qs], start=not diag, stop=True)
                    scs[j] = (sc_t, qoff)

                emit_qk(0)
                for j in range(njt):
                    sc_t, qoff = scs.pop(j)
                    jt = b * SJT + j
                    p = p_pool.tile([P, 1024], bf16, tag="p")
                    in_ap = sc_t[:].rearrange("q (h n) -> q h n", h=2)[:, :, qoff:512]
                    out_ap = p[:].rearrange("q (h n) -> q h n", h=2)[:, :, qoff:512]
                    nc.scalar.activation(out_ap, in_ap, Act.Exp, scale=SCALE)
                    if j + 1 < njt:
                        emit_qk(j + 1)
                    nc.tensor.matmul(po[0:65, qoff:512], vnat[:, jt, 0:65],
                                     p[:, qoff:512],
                                     start=(j == 0), stop=(j == njt - 1))
                    nc.tensor.matmul(po[0:65, 512 + qoff:1024], vnat[:, jt, 0:65],
                                     p[:, 512 + qoff:1024],
                                     start=(j == 0), stop=(j == njt - 1))
                    if queue:
                        queue.pop(0)()
                while queue:
                    queue.pop(0)()
                return po

            def tail_steps(b, ib, po):
                """Normalize + o_proj substeps for one finished query block."""
                q0 = b * S + ib * 512
                st = {}

                def t1():
                    dsb = nrm_pool.tile([1, 1024], f32, tag="dsb")
                    nc.scalar.activation(dsb[:], po[64:65, 0:1024], Act.Identity)
                    rsb = nrm_pool.tile([1, 1024], f32, tag="rsb")
                    if recip_exact:
                        with nc.allow_low_precision(reason="softmax denom"):
                            nc.vector.reciprocal(rsb[:], dsb[:])
                    else:
                        nc.vector.reciprocal_approx_fast(rsb[:], dsb[:])
                    rbf = nrm_pool.tile([1, 1024], bf16, tag="rbf")
                    nc.scalar.activation(rbf[:], rsb[:], Act.Identity)
                    pb = sc_pool.tile([P, 1024], f32, tag="sc")
                    nc.tensor.matmul(pb[0:64, 0:512], ones_sb[:], rbf[0:1, 0:512],
                                     start=True, stop=True)
                    nc.tensor.matmul(pb[0:64, 512:1024], ones_sb[:],
                                     rbf[0:1, 512:1024], start=True, stop=True)
                    pbs = nrm_pool.tile([64, 1024], f32, tag="pbs")
                    nc.vector.tensor_copy(pbs[:], pb[0:64, :])
                    st["pbs"] = pbs

                def t2():
                    pbs = st["pbs"]
                    at = at_pool.tile([P, 512], bf16, tag="at")
                    nc.vector.tensor_mul(at[0:64, :], po[0:64, 0:512], pbs[:, 0:512])
                    a1 = a1_pool.tile([64, 512], bf16, tag="a1")
                    nc.vector.tensor_mul(a1[:], po[0:64, 512:1024], pbs[:, 512:1024])
                    nc.sync.dma_start(at[64:128, :], a1[:])
                    st["at"] = at

                def osteps(lo, hi):
                    def s():
                        at = st["at"]
                        for k in range(lo, hi):
                            w_ps = po_pool.tile([P, 1024], f32, tag="po")
                            lhs = at[:, k * P:(k + 1) * P]
                            nc.tensor.matmul(w_ps[:, 0:512], lhs, wo_sb[:, 0:512],
                                             start=True, stop=True)
                            nc.tensor.matmul(w_ps[:, 512:1024], lhs,
                                             wo_sb[:, 512:1024],
                                             start=True, stop=True)
                            osb = osb_pool.tile([P, 1024], f32, tag="osb")
                            nc.vector.tensor_copy(osb[:], w_ps[:])
                            rows = slice(q0 + k * P, q0 + (k + 1) * P)
                            nc.sync.dma_start(out_d[rows, :], osb[:])
                    return s

                return [t1, t2, osteps(0, 1), osteps(1, 2), osteps(2, 3),
                        osteps(3, 4)]

            # --- emission: j-loops with prev-tail + next-A substeps woven in.
            # Batches alternate so a block's tail overlaps the other batch's
            # j-loop; substep order keeps the po-tag ring deadlock-free
            # (every alloc waits only on earlier-emitted releases).
            seq = [
                (0, 0, 1), (1, 0, 5), (0, 1, 2), (1, 1, 6),
                (0, 2, 3), (1, 2, 7), (0, 3, None), (1, 3, None),
            ]
            for blk in (0, 4):
                for s in a_steps(blk):
                    s()
            prev = None
            for b, ib, ablk in seq:
                queue = []
                tail = tail_steps(*prev) if prev else []
                asub = a_steps(ablk) if ablk is not None else []
                # T1, T2 first (frees the po slot), then alternate
                queue += tail[0:2]
                if asub:
                    queue += [asub[0], asub[1]]
                queue += tail[2:4]
                if asub:
                    queue += [asub[2]]
                queue += tail[4:6]
                if asub:
                    queue += asub[3:5]
                po = emit_B_jl(b, ib, queue)
                prev = (b, ib, po)
            for s in tail_steps(*prev):
                s()

    nc.compile()
    return nc


def _get_nc(mm_mode="bf16"):
    if mm_mode not in _NC_CACHE:
        _NC_CACHE[mm_mode] = _build_nc(mm_mode)
    return _NC_CACHE[mm_mode]


def make_in_maps(inputs, mm_mode="bf16"):
    """Host-side sharding/layout prep: returns the 8 per-core input dicts."""
    import ml_dtypes

    bfd = ml_dtypes.bfloat16
    hidden = np.asarray(inputs["hidden_states"], dtype=np.float32)
    pos = np.asarray(inputs["positions"])
    Wq = np.asarray(inputs["Wq"], dtype=np.float32)
    bq = np.asarray(inputs["bq"], dtype=np.float32)
    Wk = np.asarray(inputs["Wk"], dtype=np.float32)
    bk = np.asarray(inputs["bk"], dtype=np.float32)
    Wv = np.asarray(inputs["Wv"], dtype=np.float32)
    bv = np.asarray(inputs["bv"], dtype=np.float32)
    Wo = np.asarray(inputs["Wo"], dtype=np.float32)

    xT = np.ascontiguousarray(hidden.reshape(T, H).T)

    half = HD // 2
    inv = 1.0 / THETA ** (np.arange(half, dtype=np.float64) * 2.0 / HD)
    f = pos.astype(np.float64)[None, :] * inv[:, None]
    cos32 = np.cos(f)
    sin32 = np.sin(f)
    pidx = np.arange(P) % half
    sgn = np.where(np.arange(P) % HD < half, -1.0, 1.0)
    cosT = np.ascontiguousarray(cos32[pidx].astype(bfd))
    sinT = np.ascontiguousarray((sin32[pidx] * sgn[:, None]).astype(bfd))

    m = np.arange(P)
    sig = np.where(m % HD < half, m + half, m - half)
    perm = np.zeros((P, P), np.float32)
    perm[sig, m] = 1.0
    ident = np.eye(P, dtype=np.float32)
    maskc = np.where(np.arange(P)[:, None] > np.arange(P)[None, :], -1e9, 0.0)
    ones64 = np.ones((1, 64), np.float32)
    onescol = np.ones((P, 2 * SJT), np.float32)

    xTa = xT.astype(bfd)
    in_maps = []
    for c in range(NCORES):
        g = c // 2
        wkv = np.ascontiguousarray(
            np.concatenate(
                [Wk[:, g * HD:(g + 1) * HD], Wv[:, g * HD:(g + 1) * HD]], axis=1
            )
        )
        bkv = np.ascontiguousarray(
            np.concatenate([bk[g * HD:(g + 1) * HD], bv[g * HD:(g + 1) * HD]])[:, None]
        )
        in_maps.append({
            "xT": xTa,
            "wq": np.ascontiguousarray(Wq[:, c * P:(c + 1) * P]).astype(bfd),
            "bq": np.ascontiguousarray(bq[c * P:(c + 1) * P][:, None]),
            "wkv": wkv.astype(bfd),
            "bkv": bkv,
            "wo": np.ascontiguousarray(Wo[c * P:(c + 1) * P, :]).astype(bfd),
            "cosT": cosT,
            "sinT": sinT,
            "perm": perm.astype(bfd),
            "ident": ident.astype(bfd),
            "maskc": maskc.astype(bfd),
            "ones64": ones64.astype(bfd),
            "ones64f": ones64,
            "onescol": onescol.astype(bfd),
        })
    return in_maps


def kernel(**inputs):
    global LAST_RESULT
    from concourse.bass_utils import run_bass_kernel_spmd

    mm_mode = os.environ.get("KERNEL_MM_MODE", "bf16")
    nc = _get_nc(mm_mode)
    in_maps = make_in_maps(inputs, mm_mode)
    res = run_bass_kernel_spmd(nc, in_maps, core_ids=list(range(NCORES)))
    LAST_RESULT = res
    out = res.results[0]["out"].astype(np.float32, copy=True)
    for rr in res.results[1:]:
        out += rr["out"]
    return out.reshape(B, S, H)


# revision 26
# speedup vs baseline: 1.0590x; 1.0590x over previous
"""Trainium2 Bass kernel for MiMoAudio attention (GQA + neox RoPE + causal softmax + o_proj).

Strategy (tensor-parallel over heads, 8 cores):
  - Each core owns 2 of the 16 q heads (128 q channels) and the single kv head
    (64 channels) that those q heads attend to (GQA group).
  - Host pre-transposes hidden_states to xT [H, B*S]; all activations live
    feature-on-partitions: qT [128, T], kvT [128, T] (k rows 0:64, v 64:128),
    scores [keys=128, queries] per 128-key tile.
  - Per j-tile both heads' scores land in one [128,1024] PSUM tile (the QK
    pair runs concurrently via PE row-tiling at tile_position (0,0)/(64,0)),
    one Exp activation covers both, and the PV pair accumulates into a shared
    [128,1024] po tile (rows 0:64 = out, row 64 = ones-row denominator).
  - Causal masking: query-range restriction per diagonal tile plus a -1e9
    lower-triangle [128,128] constant accumulated into the scores via a tiny
    extra matmul (N=128) before the exp.
  - Softmax denominators: ACT copies the denom row to SBUF, DVE
    reciprocal_approx_fast, broadcast back over 64 partitions via a K=1
    matmul, DVE multiply.
  - o_proj row-slice per core produces a partial [T, H] output; host sums the
    8 partials (the TP all-reduce, done at unshard time).
  - Emission order interleaves projection blocks and the two batches'
    attention blocks so ACT (exp) stays saturated and PE stays warm.
"""

import os
import numpy as np

B = 2
S = 2048
T = B * S
H = 1024
HD = 64
P = 128
NCORES = 8
THETA = 10000.0
SCALE = HD ** -0.5
NBLK = T // 512
HO = H // P
SJT = S // P

_NC_CACHE = {}
LAST_RESULT = None


def _ensure_ntff_hook():
    """Provide antenv.axon_hooks if the image lacks it, so BASS_TRACE=1
    profiling works under axon instead of crashing on import."""
    import sys
    import types

    try:
        import antenv.axon_hooks  # noqa: F401
        return
    except ImportError:
        pass
    mod = types.ModuleType("antenv.axon_hooks")
    mod._hook = None

    def set_axon_ntff_profile_hook(h):
        mod._hook = h

    def get_axon_ntff_profile_hook():
        return mod._hook

    mod.set_axon_ntff_profile_hook = set_axon_ntff_profile_hook
    mod.get_axon_ntff_profile_hook = get_axon_ntff_profile_hook
    sys.modules["antenv.axon_hooks"] = mod
    try:
        import antenv

        antenv.axon_hooks = mod
    except ImportError:
        pass
    try:
        from trn_agent_boot.trn_boot import _ntff_profile_via_ctypes

        hook = _ntff_profile_via_ctypes("/opt/axon/libaxon_pjrt.so")
        if hook is not None:
            mod.set_axon_ntff_profile_hook(hook)
    except Exception:
        pass


_ensure_ntff_hook()


def _build_nc(mm_mode="bf16"):
    import concourse.bass as bass  # noqa: F401
    import concourse.mybir as mybir
    import concourse.tile as tile
    from concourse import bacc

    f32 = mybir.dt.float32
    bf16 = mybir.dt.bfloat16
    Act = mybir.ActivationFunctionType
    recip_exact = os.environ.get("KERNEL_RECIP", "approx") == "exact"

    nc = bacc.Bacc(None, target_bir_lowering=False, debug=False)

    xT_d = nc.dram_tensor("xT", [H, T], bf16, kind="ExternalInput")
    wq_d = nc.dram_tensor("wq", [H, P], bf16, kind="ExternalInput")
    bq_d = nc.dram_tensor("bq", [P, 1], f32, kind="ExternalInput")
    wkv_d = nc.dram_tensor("wkv", [H, P], bf16, kind="ExternalInput")
    bkv_d = nc.dram_tensor("bkv", [P, 1], f32, kind="ExternalInput")
    wo_d = nc.dram_tensor("wo", [P, H], bf16, kind="ExternalInput")
    cos_d = nc.dram_tensor("cosT", [P, S], bf16, kind="ExternalInput")
    sin_d = nc.dram_tensor("sinT", [P, S], bf16, kind="ExternalInput")
    perm_d = nc.dram_tensor("perm", [P, P], bf16, kind="ExternalInput")
    id_d = nc.dram_tensor("ident", [P, P], bf16, kind="ExternalInput")
    mask_d = nc.dram_tensor("maskc", [P, P], bf16, kind="ExternalInput")
    ones_d = nc.dram_tensor("ones64", [1, 64], bf16, kind="ExternalInput")
    onesf_d = nc.dram_tensor("ones64f", [1, 64], f32, kind="ExternalInput")
    onescol_d = nc.dram_tensor("onescol", [P, 2 * SJT], bf16, kind="ExternalInput")
    out_d = nc.dram_tensor("out", [T, H], f32, kind="ExternalOutput")

    with tile.TileContext(nc) as tc:
        with (
            tc.tile_pool(name="const", bufs=1) as cpool,
            tc.tile_pool(name="persist", bufs=1) as ppool,
            tc.tile_pool(name="xt", bufs=3) as xt_pool,
            tc.tile_pool(name="ptile", bufs=4) as p_pool,
            tc.tile_pool(name="ropetmp", bufs=3) as tmp_pool,
            tc.tile_pool(name="atp", bufs=2) as at_pool,
            tc.tile_pool(name="a1p", bufs=2) as a1_pool,
            tc.tile_pool(name="nrm", bufs=2) as nrm_pool,
            tc.tile_pool(name="osb", bufs=3) as osb_pool,
            tc.tile_pool(name="sc_ps", bufs=2, space="PSUM") as sc_pool,
            tc.tile_pool(name="po_ps", bufs=2, space="PSUM") as po_pool,
        ):
            # --- constants (wq/wkv + the first two x-blocks first: they gate
            # the first matmuls; everything else loads behind them) ----------
            wq_sb = cpool.tile([P, HO, P], bf16)
            nc.sync.dma_start(wq_sb[:], wq_d[:].rearrange("(o p) m -> p o m", p=P))
            wkv_sb = cpool.tile([P, HO, P], bf16)
            nc.sync.dma_start(wkv_sb[:], wkv_d[:].rearrange("(o p) m -> p o m", p=P))
            xT_r = xT_d[:].rearrange("(o p) t -> p o t", p=P)
            xt_first = {}
            for blk in (0, 4):
                xt = xt_pool.tile([P, HO, 512], bf16, tag="xt")
                nc.sync.dma_start(
                    xt[:], xT_r[:, :, blk * 512:(blk + 1) * 512]
                )
                xt_first[blk] = xt
            bq_sb = cpool.tile([P, 1], f32)
            nc.sync.dma_start(bq_sb[:], bq_d[:])
            bkv_sb = cpool.tile([P, 1], f32)
            nc.sync.dma_start(bkv_sb[:], bkv_d[:])
            wo_sb = cpool.tile([P, H], bf16)
            nc.sync.dma_start(wo_sb[:], wo_d[:])
            cos_sb = cpool.tile([P, S], bf16)
            nc.sync.dma_start(cos_sb[:], cos_d[:])
            sin_sb = cpool.tile([P, S], bf16)
            nc.sync.dma_start(sin_sb[:], sin_d[:])
            perm_sb = cpool.tile([P, P], bf16)
            nc.sync.dma_start(perm_sb[:], perm_d[:])
            id_sb = cpool.tile([P, P], bf16)
            nc.sync.dma_start(id_sb[:], id_d[:])
            mask_sb = cpool.tile([P, P], bf16)
            nc.sync.dma_start(mask_sb[:], mask_d[:])
            ones_sb = cpool.tile([1, 64], bf16)
            nc.sync.dma_start(ones_sb[:], ones_d[:])
            onesf_sb = cpool.tile([1, 64], f32)
            nc.sync.dma_start(onesf_sb[:], onesf_d[:])

            # --- persistent activations -------------------------------------
            qT = ppool.tile([P, T], bf16)
            kvT = ppool.tile([P, T], bf16)
            khi = ppool.tile([P, T], bf16)
            vnat = ppool.tile([P, 2 * SJT, 72], bf16)
            nc.sync.dma_start(
                vnat[:, :, 64:65], onescol_d[:].rearrange("p (j o) -> p j o", o=1)
            )

            def a_steps(blk):
                """QKV projection + rope + layout prep for one 512-token block,
                as a list of substep closures for interleaving into j-loops.
                All PSUM tiles come from the po-tag ring so the sc-tag ring
                stays dedicated to QK score tiles."""
                tb = slice(blk * 512, (blk + 1) * 512)
                st = {}

                def s1():
                    if blk in xt_first:
                        xt = xt_first.pop(blk)
                    else:
                        xt = xt_pool.tile([P, HO, 512], bf16, tag="xt")
                        nc.sync.dma_start(xt[:], xT_r[:, :, tb])
                    qkv = po_pool.tile([P, 1024], f32, tag="po")
                    for o in range(HO):
                        nc.tensor.matmul(
                            qkv[:, 0:512], wq_sb[:, o, :], xt[:, o, :],
                            start=(o == 0), stop=(o == HO - 1),
                        )
                    st["xt"], st["qkv"] = xt, qkv

                def s2():
                    xt, qkv = st["xt"], st["qkv"]
                    for o in range(HO):
                        nc.tensor.matmul(
                            qkv[:, 512:1024], wkv_sb[:, o, :], xt[:, o, :],
                            start=(o == 0), stop=(o == HO - 1),
                        )
                    nc.vector.tensor_scalar_add(qT[:, tb], qkv[:, 0:512], bq_sb[:])
                    nc.vector.tensor_scalar_add(kvT[:, tb], qkv[:, 512:1024],
                                                bkv_sb[:])

                def s3():
                    sc0 = (blk * 512) % S
                    ss = slice(sc0, sc0 + 512)
                    pr = po_pool.tile([P, 1024], f32, tag="po")
                    nc.tensor.matmul(pr[:, 0:512], perm_sb[:], qT[:, tb],
                                     start=True, stop=True)
                    nc.tensor.matmul(pr[0:64, 512:1024], perm_sb[0:64, 0:64],
                                     kvT[0:64, tb], start=True, stop=True)
                    tmp = tmp_pool.tile([P, 512], bf16, tag="ropetmp")
                    nc.vector.tensor_mul(tmp[:], pr[:, 0:512], sin_sb[:, ss])
                    nc.vector.tensor_mul(qT[:, tb], qT[:, tb], cos_sb[:, ss])
                    nc.vector.tensor_add(qT[:, tb], qT[:, tb], tmp[:])
                    tmpk = tmp_pool.tile([P, 512], bf16, tag="ropetmp")
                    nc.vector.tensor_mul(tmpk[0:64, :], pr[0:64, 512:1024],
                                         sin_sb[0:64, ss])
                    nc.vector.tensor_mul(kvT[0:64, tb], kvT[0:64, tb],
                                         cos_sb[0:64, ss])
                    nc.vector.tensor_add(kvT[0:64, tb], kvT[0:64, tb],
                                         tmpk[0:64, :])
                    nc.sync.dma_start(khi[64:128, tb], kvT[0:64, tb])

                def vsteps(lo, hi):
                    def s():
                        for jj in range(lo, hi):
                            jt = blk * 4 + jj
                            tp = po_pool.tile([P, 1024], bf16, tag="po")
                            nc.tensor.transpose(
                                tp[:, 0:64], kvT[64:128, jt * P:(jt + 1) * P],
                                id_sb[64:128, 64:128],
                            )
                            nc.vector.tensor_copy(vnat[:, jt, 0:64], tp[:, 0:64])
                    return s

                return [s1, s2, s3, vsteps(0, 2), vsteps(2, 4)]

            def emit_B_jl(b, ib, queue):
                """Attention j-loop; pops one interleave substep per j."""
                njt = 4 * (ib + 1)
                q0 = b * S + ib * 512
                po = po_pool.tile([P, 1024], f32, tag="po")
                scs = {}

                def emit_qk(j):
                    qoff = max(0, 128 * (j - 4 * ib))
                    jt = b * SJT + j
                    js = slice(jt * P, (jt + 1) * P)
                    qs = slice(q0 + qoff, q0 + 512)
                    sc_t = sc_pool.tile([P, 1024], f32, tag="sc")
                    diag = j >= 4 * ib
                    if diag:
                        # mask first (start=True clears the bank, writes the
                        # -1e9 triangle); QK then accumulates onto it, so the
                        # exp's dependency ends at the QK matmul itself.
                        nc.tensor.matmul(sc_t[:, qoff:qoff + 128], id_sb[:],
                                         mask_sb[:], start=True, stop=False)
                        nc.tensor.matmul(sc_t[:, 512 + qoff:512 + qoff + 128],
                                         id_sb[:], mask_sb[:],
                                         start=True, stop=False)
                    nc.tensor.matmul(sc_t[:, qoff:512], kvT[0:64, js],
                                     qT[0:64, qs], start=not diag, stop=True)
                    nc.tensor.matmul(sc_t[:, 512 + qoff:1024], khi[64:128, js],
                                     qT[64:128, qs], start=not diag, stop=True)
                    scs[j] = (sc_t, qoff)

                emit_qk(0)
                for j in range(njt):
                    sc_t, qoff = scs.pop(j)
                    jt = b * SJT + j
                    p = p_pool.tile([P, 1024], bf16, tag="p")
                    in_ap = sc_t[:].rearrange("q (h n) -> q h n", h=2)[:, :, qoff:512]
                    out_ap = p[:].rearrange("q (h n) -> q h n", h=2)[:, :, qoff:512]
                    nc.scalar.activation(out_ap, in_ap, Act.Exp, scale=SCALE)
                    if j + 1 < njt:
                        emit_qk(j + 1)
                    nc.tensor.matmul(po[0:65, qoff:512], vnat[:, jt, 0:65],
                                     p[:, qoff:512],
                                     start=(j == 0), stop=(j == njt - 1))
                    nc.tensor.matmul(po[0:65, 512 + qoff:1024], vnat[:, jt, 0:65],
                                     p[:, 512 + qoff:1024],
                                     start=(j == 0), stop=(j == njt - 1))
                    if queue:
                        queue.pop(0)()
                while queue:
                    queue.pop(0)()
                return po

            def tail_steps(b, ib, po):
                """Normalize + o_proj substeps for one finished query block."""
                q0 = b * S + ib * 512
                st = {}

                def t1():
                    dsb = nrm_pool.tile([1, 1024], f32, tag="dsb")
                    nc.scalar.activation(dsb[:], po[64:65, 0:1024], Act.Identity)
                    rsb = nrm_pool.tile([1, 1024], f32, tag="rsb")
                    if recip_exact:
                        with nc.allow_low_precision(reason="softmax denom"):
                            nc.vector.reciprocal(rsb[:], dsb[:])
                    else:
                        nc.vector.reciprocal_approx_fast(rsb[:], dsb[:])
                    pbs = nrm_pool.tile([64, 1024], f32, tag="pbs")
                    nc.gpsimd.partition_broadcast(pbs[:], rsb[:])
                    st["pbs"] = pbs

                def t2():
                    pbs = st["pbs"]
                    at = at_pool.tile([P, 512], bf16, tag="at")
                    nc.vector.tensor_mul(at[0:64, :], po[0:64, 0:512], pbs[:, 0:512])
                    a1 = a1_pool.tile([64, 512], bf16, tag="a1")
                    nc.vector.tensor_mul(a1[:], po[0:64, 512:1024], pbs[:, 512:1024])
                    nc.sync.dma_start(at[64:128, :], a1[:])
                    st["at"] = at

                def osteps(lo, hi):
                    def s():
                        at = st["at"]
                        for k in range(lo, hi):
                            w_ps = po_pool.tile([P, 1024], f32, tag="po")
                            lhs = at[:, k * P:(k + 1) * P]
                            nc.tensor.matmul(w_ps[:, 0:512], lhs, wo_sb[:, 0:512],
                                             start=True, stop=True)
                            nc.tensor.matmul(w_ps[:, 512:1024], lhs,
                                             wo_sb[:, 512:1024],
                                             start=True, stop=True)
                            osb = osb_pool.tile([P, 1024], f32, tag="osb")
                            nc.vector.tensor_copy(osb[:], w_ps[:])
                            rows = slice(q0 + k * P, q0 + (k + 1) * P)
                            nc.sync.dma_start(out_d[rows, :], osb[:])
                    return s

                return [t1, t2, osteps(0, 1), osteps(1, 2), osteps(2, 3),
                        osteps(3, 4)]

            # --- emission: j-loops with prev-tail + next-A substeps woven in.
            # Batches alternate so a block's tail overlaps the other batch's
            # j-loop; substep order keeps the po-tag ring deadlock-free
            # (every alloc waits only on earlier-emitted releases).
            seq = [
                (0, 0, 1), (1, 0, 5), (0, 1, 2), (1, 1, 6),
                (0, 2, 3), (1, 2, 7), (0, 3, None), (1, 3, None),
            ]
            for blk in (0, 4):
                for s in a_steps(blk):
                    s()
            prev = None
            for b, ib, ablk in seq:
                queue = []
                tail = tail_steps(*prev) if prev else []
                asub = a_steps(ablk) if ablk is not None else []
                # T1, T2 first (frees the po slot), then alternate
                queue += tail[0:2]
                if asub:
                    queue += [asub[0], asub[1]]
                queue += tail[2:4]
                if asub:
                    queue += [asub[2]]
                queue += tail[4:6]
                if asub:
                    queue += asub[3:5]
                po = emit_B_jl(b, ib, queue)
                prev = (b, ib, po)
            for s in tail_steps(*prev):
                s()

    nc.compile()
    return nc


def _get_nc(mm_mode="bf16"):
    if mm_mode not in _NC_CACHE:
        _NC_CACHE[mm_mode] = _build_nc(mm_mode)
    return _NC_CACHE[mm_mode]


def make_in_maps(inputs, mm_mode="bf16"):
    """Host-side sharding/layout prep: returns the 8 per-core input dicts."""
    import ml_dtypes

    bfd = ml_dtypes.bfloat16
    hidden = np.asarray(inputs["hidden_states"], dtype=np.float32)
    pos = np.asarray(inputs["positions"])
    Wq = np.asarray(inputs["Wq"], dtype=np.float32)
    bq = np.asarray(inputs["bq"], dtype=np.float32)
    Wk = np.asarray(inputs["Wk"], dtype=np.float32)
    bk = np.asarray(inputs["bk"], dtype=np.float32)
    Wv = np.asarray(inputs["Wv"], dtype=np.float32)
    bv = np.asarray(inputs["bv"], dtype=np.float32)
    Wo = np.asarray(inputs["Wo"], dtype=np.float32)

    xT = np.ascontiguousarray(hidden.reshape(T, H).T)

    half = HD // 2
    inv = 1.0 / THETA ** (np.arange(half, dtype=np.float64) * 2.0 / HD)
    f = pos.astype(np.float64)[None, :] * inv[:, None]
    cos32 = np.cos(f)
    sin32 = np.sin(f)
    pidx = np.arange(P) % half
    sgn = np.where(np.arange(P) % HD < half, -1.0, 1.0)
    cosT = np.ascontiguousarray(cos32[pidx].astype(bfd))
    sinT = np.ascontiguousarray((sin32[pidx] * sgn[:, None]).astype(bfd))

    m = np.arange(P)
    sig = np.where(m % HD < half, m + half, m - half)
    perm = np.zeros((P, P), np.float32)
    perm[sig, m] = 1.0
    ident = np.eye(P, dtype=np.float32)
    maskc = np.where(np.arange(P)[:, None] > np.arange(P)[None, :], -1e9, 0.0)
    ones64 = np.ones((1, 64), np.float32)
    onescol = np.ones((P, 2 * SJT), np.float32)

    xTa = xT.astype(bfd)
    in_maps = []
    for c in range(NCORES):
        g = c // 2
        wkv = np.ascontiguousarray(
            np.concatenate(
                [Wk[:, g * HD:(g + 1) * HD], Wv[:, g * HD:(g + 1) * HD]], axis=1
            )
        )
        bkv = np.ascontiguousarray(
            np.concatenate([bk[g * HD:(g + 1) * HD], bv[g * HD:(g + 1) * HD]])[:, None]
        )
        in_maps.append({
            "xT": xTa,
            "wq": np.ascontiguousarray(Wq[:, c * P:(c + 1) * P]).astype(bfd),
            "bq": np.ascontiguousarray(bq[c * P:(c + 1) * P][:, None]),
            "wkv": wkv.astype(bfd),
            "bkv": bkv,
            "wo": np.ascontiguousarray(Wo[c * P:(c + 1) * P, :]).astype(bfd),
            "cosT": cosT,
            "sinT": sinT,
            "perm": perm.astype(bfd),
            "ident": ident.astype(bfd),
            "maskc": maskc.astype(bfd),
            "ones64": ones64.astype(bfd),
            "ones64f": ones64,
            "onescol": onescol.astype(bfd),
        })
    return in_maps


def kernel(**inputs):
    global LAST_RESULT
    from concourse.bass_utils import run_bass_kernel_spmd

    mm_mode = os.environ.get("KERNEL_MM_MODE", "bf16")
    nc = _get_nc(mm_mode)
    in_maps = make_in_maps(inputs, mm_mode)
    res = run_bass_kernel_spmd(nc, in_maps, core_ids=list(range(NCORES)))
    LAST_RESULT = res
    out = res.results[0]["out"].astype(np.float32, copy=True)
    for rr in res.results[1:]:
        out += rr["out"]
    return out.reshape(B, S, H)
